# revision 68
# baseline (speedup 1.0000x reference)
"""FAGCN (FAConv x3) Trainium2 kernel, 8-core SPMD — v3 (src-sharded).

Sharding: nodes partitioned across 8 cores (6250 each, padded to 6272);
edges assigned to the owner of SRC and sorted by (half, global dst
64-window). Each core's gathers read only its LOCAL per-node table
(rows [h(128 bf16) | al | pad] = 512B; the gather DMA cost model
charges the same for any row <= 512B, so al rides free; SWDGE calls
are capped at 1024 descriptors -- 2048 hangs the device). v2's 25.7MB
table AllGather is gone; per layer the cross-core traffic is
  - two ~20KB-ar AllGathers (att_r . h halves, bf16), and
  - two ReduceScatters of bf16 message partials [feat, node] with flat
    1-D APs (fake_nrt mangles multi-level collective APs). The split is
    128-window aligned and ASYMMETRIC (4096/2176, tuned wa128=32: a
    bigger half A shrinks the exposed RS-B tail while RS-A still
    hides): RS-A fires when every core's first 4096 nodes have
    spilled (~65% through the edge pass) and hides
    behind the remaining chunks; collectives serialize on the single
    COLLECTIVE_CORES resource, so only RS-B (+AG) sits exposed in the
    per-layer tail.
Per 32-chunk gather call (4x 1024-descriptor SWDGE sub-calls; ~3 calls
in flight -- wider calls amortize the per-call alRow/arWin DMAs and
chain latency, worth ~150k ns over 16-chunk calls) the edge pass
builds tanh input in PSUM with
PE rank-1s (al columns of the gathered rows PE-transposed, Act-copied,
then DMA-flattened onto partition 0 -- matmul operands need base
partition 0/32/64; ar windows come from a per-chunk-ordered DRAM
staging of the AllGather) and runs one wide tanh per 8-chunk group
(Act ops pay ~185ns fixed overhead). The symmetric gcn norm
(dinv_src*dinv_dst, static) is folded into the one-hot via a two-op
tensor_scalar (is_equal, mult), so the table keeps raw h and
h_new = rs_out + EPS*h0 directly. Segment sums accumulate in 8-window
PSUM blocks, evacuate (Act/DVE alternating) into per-section staging,
and spill with one DMA per (half, section) -- DMAs cost ~630ns of
serial HWDGE each, so batching counts matters everywhere (merged
table stores, staged emb/lsm outputs).
"""
import ml_dtypes
import numpy as np

import concourse.bacc as bacc
import concourse.bass as bass
import concourse.mybir as mybir
import concourse.tile as tile
from concourse.bass_utils import run_bass_kernel_spmd
from concourse.masks import make_identity

F32 = mybir.dt.float32
BF16 = mybir.dt.bfloat16
I16 = mybir.dt.int16

EPS = 0.1


class Cfg:
    def __init__(self, n_nodes, n_edges, in_dim, out_dim, n_layers,
                 n_cores=8, csup=32, spill_f32=False, wa128=0):
        self.SPILL_F32 = spill_f32
        self.N = n_nodes
        self.E = n_edges
        self.IN = in_dim
        self.H = 128
        self.OUT = out_dim
        self.NL = n_layers
        self.NC = n_cores
        self.NV = n_nodes // n_cores          # owned nodes per core
        assert self.NV * n_cores == n_nodes
        self.W = (self.NV + 127) // 128       # 128-node windows per core
        self.NP = self.W * 128                # padded nodes per core
        self.NPG = self.NP * n_cores          # padded global nodes
        self.KT = in_dim // 128
        assert in_dim % 128 == 0
        self.DW = 64                          # dst-window width
        self.WL = self.NP // self.DW          # local 64-windows per core
        self.WA128 = wa128 if wa128 else (self.W + 1) // 2  # 128-windows in half A
        self.WHA = self.WA128 * 2             # 64-windows in half A
        self.WHB = self.WL - self.WHA
        self.NPHA = self.WHA * self.DW        # nodes in half A
        self.NPHB = self.NP - self.NPHA
        self.NWG = self.WL * n_cores          # global 64-windows
        self.CSUP = csup                      # chunks per gather call
        self.AB = 4
        self.GPB = 3                          # gather pool bufs
        self.RWE = 256                        # table row elems (512B)
        self.BPW = 8                          # windows per PSUM spill block
        # arc2 layout: window gw -> (partition gw//ACB, col (gw%ACB)*DW)
        self.ACB = (self.NWG + 127) // 128    # col-blocks per partition
        self.DDSS = 16384  # 1024-desc SWDGE ring per sub-call


FULL = Cfg(50000, 600000, 512, 64, 3, wa128=32)


# ----------------------------------------------------------------- planner

def plan_edges(cfg, edge_index):
    """Host-side edge sharding by SRC owner; chunk schedule shared by all
    cores (max chunk count per window)."""
    src = edge_index[0].astype(np.int64)
    dst = edge_index[1].astype(np.int64)
    owner = src // cfg.NV
    srow = src % cfg.NV                         # local table row
    g = (dst // cfg.NV) * cfg.NP + dst % cfg.NV  # padded global node
    gw = g // cfg.DW                             # global 64-window
    rel = (g % cfg.DW).astype(np.float32)
    wloc = (dst % cfg.NV) // cfg.DW              # local 64-window of dst
    half = (wloc >= cfg.WHA).astype(np.int64)

    # window processing order: half-major, then gw ascending
    worder = sorted(range(cfg.NWG), key=lambda w: ((w % cfg.WL) >= cfg.WHA, w))
    wpos = np.empty(cfg.NWG, np.int64)
    for i, w in enumerate(worder):
        wpos[w] = i

    counts = np.zeros((cfg.NC, cfg.NWG), np.int64)
    per_core = []
    for c in range(cfg.NC):
        m = owner == c
        s_r, gw_c, rel_c, g_c = srow[m], gw[m], rel[m], g[m]
        order = np.lexsort((g_c, wpos[gw_c]))
        s_r, gw_c, rel_c = s_r[order], gw_c[order], rel_c[order]
        cnt = np.bincount(gw_c, minlength=cfg.NWG)
        counts[c] = cnt
        per_core.append((m, order, s_r, gw_c, rel_c))

    nch = np.maximum((counts.max(axis=0) + 127) // 128, 1)  # [NWG] chunks
    # chunk meta in window order: (gw, first_of_window, last_of_window)
    chunk_meta = []
    for w in worder:
        n = int(nch[w])
        for k in range(n):
            chunk_meta.append((w, k == 0, k == n - 1))
    NCH = len(chunk_meta)
    EPAD = NCH * 128

    # per-window chunk start offsets (in chunk index)
    wstart = {}
    pos = 0
    for w in worder:
        wstart[w] = pos
        pos += int(nch[w])

    deg = np.bincount(dst, minlength=cfg.N).astype(np.float64)
    dinv = np.where(deg > 0, 1.0 / np.sqrt(np.maximum(deg, 1.0)), 0.0)
    norm_e = (dinv[src] * dinv[dst]).astype(np.float32)

    cores = []
    for c in range(cfg.NC):
        m, order, s_r, gw_c, rel_c = per_core[c]
        nrm = norm_e[m][order]
        gidx = np.zeros(EPAD, np.int64)
        relp = np.full(EPAD, 999.0, np.float32)
        nrmp = np.zeros(EPAD, np.float32)
        ptr = 0
        for w in worder:
            n = int(counts[c, w])
            base = wstart[w] * 128
            gidx[base:base + n] = s_r[ptr:ptr + n]
            relp[base:base + n] = rel_c[ptr:ptr + n]
            nrmp[base:base + n] = nrm[ptr:ptr + n]
            ptr += n
        assert ptr == len(s_r)

        def wrap16(v):
            a = v.astype(np.int16).reshape(-1, 16).T.copy()
            return np.tile(a, (8, 1))

        def lanes(v):
            return v.reshape(-1, 128).T.copy()

        cores.append(dict(gidx=wrap16(gidx), rel=lanes(relp), norme=lanes(nrmp)))
    return dict(nch=nch, NCH=NCH, EPAD=EPAD, chunk_meta=chunk_meta,
                worder=worder, cores=cores)


def call_schedule(cfg, plan):
    """Gather calls of <=CSUP chunks."""
    NCH = plan["NCH"]
    calls = []
    c0 = 0
    while c0 < NCH:
        n = min(cfg.CSUP, NCH - c0)
        calls.append((c0, n))
        c0 += n
    return calls


def shard_inputs(cfg, inputs, plan):
    x = np.asarray(inputs["x"], np.float32)
    t1_w = np.asarray(inputs["t1_w"], np.float32)
    t1_b = np.asarray(inputs["t1_b"], np.float32)
    t2_w = np.asarray(inputs["t2_w"], np.float32)
    t2_b = np.asarray(inputs["t2_b"], np.float32)
    att_l = np.asarray(inputs["att_l"], np.float32)
    att_r = np.asarray(inputs["att_r"], np.float32)

    bf = ml_dtypes.bfloat16
    w1t = t1_w.T.astype(bf)
    w1t_tiles = w1t.reshape(cfg.KT, 128, cfg.H)
    b1col = t1_b[:, None].copy()
    attlr = np.zeros((128, cfg.NL * 2), np.float32)
    for i in range(cfg.NL):
        attlr[:, 2 * i] = att_l[i]
        attlr[:, 2 * i + 1] = att_r[i]
    attlr = attlr.astype(bf)
    t2wt = t2_w.T.astype(bf)
    b2rep = np.broadcast_to(t2_b, (128, cfg.OUT)).copy()
    iota = np.tile(np.arange(cfg.DW, dtype=np.float32), (128, 1)).astype(bf)

    in_maps = []
    for c in range(cfg.NC):
        lo = c * cfg.NV
        xc = np.zeros((cfg.NP, cfg.IN), np.float32)
        xc[:cfg.NV] = x[lo:lo + cfg.NV]
        xt = (xc.reshape(cfg.W, 128, cfg.KT, 128).transpose(3, 0, 2, 1)
              .astype(bf))
        pc = plan["cores"][c]
        in_maps.append(dict(
            xt=xt, w1t=w1t_tiles, b1col=b1col, attlr=attlr,
            t2wt=t2wt, b2rep=b2rep, iota=iota,
            gidx=pc["gidx"], rel=pc["rel"], norme=pc["norme"],
        ))
    return in_maps


# ----------------------------------------------------------------- builder

def build_program(cfg, plan, skip=frozenset(), dbg=False):
    NCH = plan["NCH"]
    meta = plan["chunk_meta"]
    EPAD = plan["EPAD"]
    worder = plan["worder"]
    W = cfg.W
    CS = cfg.CSUP
    DW = cfg.DW
    RWE = cfg.RWE
    calls = call_schedule(cfg, plan)

    nc = bacc.Bacc("TRN2", target_bir_lowering=False, debug=False,
                   num_devices=cfg.NC, num_swdge_queues=2,
                   dynamic_dma_scratch_size=cfg.DDSS)

    # ---- I/O
    t_xt = nc.dram_tensor("xt", [128, W, cfg.KT, 128], BF16, kind="ExternalInput")
    t_w1t = nc.dram_tensor("w1t", [cfg.KT, 128, cfg.H], BF16, kind="ExternalInput")
    t_b1 = nc.dram_tensor("b1col", [cfg.H, 1], F32, kind="ExternalInput")
    t_att = nc.dram_tensor("attlr", [128, cfg.NL * 2], BF16, kind="ExternalInput")
    t_t2 = nc.dram_tensor("t2wt", [cfg.H, cfg.OUT], BF16, kind="ExternalInput")
    t_b2 = nc.dram_tensor("b2rep", [128, cfg.OUT], F32, kind="ExternalInput")
    t_iota = nc.dram_tensor("iota", [128, cfg.DW], BF16, kind="ExternalInput")
    t_gidx = nc.dram_tensor("gidx", [128, EPAD // 16], I16, kind="ExternalInput")
    t_rel = nc.dram_tensor("rel", [128, NCH], F32, kind="ExternalInput")
    t_nrm = nc.dram_tensor("norme", [128, NCH], F32, kind="ExternalInput")
    t_lsm = nc.dram_tensor("lsm", [cfg.NP, cfg.OUT], F32, kind="ExternalOutput")
    t_emb = nc.dram_tensor("emb", [cfg.NP, cfg.OUT], F32, kind="ExternalOutput")
    t_hdbg = (nc.dram_tensor("hdbg", [cfg.NL + 1, 128, cfg.NP], F32,
                             kind="ExternalOutput") if dbg else None)
    t_tabdbg = (nc.dram_tensor("tabdbg", [cfg.NP, RWE], BF16,
                               kind="ExternalOutput") if dbg else None)
    t_ardbg = (nc.dram_tensor("ardbg", [NCH * cfg.DW], BF16,
                              kind="ExternalOutput") if dbg else None)
    t_padbg = (nc.dram_tensor("padbg", [cfg.NC * 128 * cfg.NPHA],
                              F32 if cfg.SPILL_F32 else BF16,
                              kind="ExternalOutput") if dbg else None)
    t_pbdbg = (nc.dram_tensor("pbdbg", [cfg.NC * 128 * cfg.NPHB],
                              F32 if cfg.SPILL_F32 else BF16,
                              kind="ExternalOutput") if dbg else None)
    t_rsdbg = (nc.dram_tensor("rsdbg", [128, cfg.NP],
                              F32 if cfg.SPILL_F32 else BF16,
                              kind="ExternalOutput") if dbg else None)

    # ---- internal DRAM
    d_tab0 = nc.dram_tensor("tab_loc0", [cfg.NP, RWE], BF16)
    d_tabs = [d_tab0, d_tab0]
    d_ar_loc = nc.dram_tensor("ar_loc", [cfg.NP], BF16)
    d_ar_fullA = nc.dram_tensor("ar_fullA", [cfg.NC * cfg.NPHA], BF16,
                                addr_space="Shared")
    d_ar_fullB = nc.dram_tensor("ar_fullB", [cfg.NC * cfg.NPHB], BF16,
                                addr_space="Shared")
    d_ar_chunk = nc.dram_tensor("ar_chunk", [NCH * cfg.DW], BF16)
    # flat per-half partials: [NC, 128, NPH] contiguous
    SPDT = F32 if cfg.SPILL_F32 else BF16
    NPHS = (cfg.NPHA, cfg.NPHB)
    d_partA = nc.dram_tensor("partA", [cfg.NC * 128 * cfg.NPHA], SPDT)
    d_partB = nc.dram_tensor("partB", [cfg.NC * 128 * cfg.NPHB], SPDT)
    d_rsA = nc.dram_tensor("rsA", [128 * cfg.NPHA], SPDT)
    d_rsB = nc.dram_tensor("rsB", [128 * cfg.NPHB], SPDT)

    rg = [list(range(cfg.NC))]

    # window -> chunk span, block schedule
    nch = plan["nch"]
    # spill blocks: consecutive windows in worder, grouped per (half, sect):
    blocks = []   # (half, [windows])
    for h in range(2):
        for sect in range(cfg.NC):
            ws = [w for w in worder
                  if ((w % cfg.WL) >= cfg.WHA) == bool(h)
                  and w // cfg.WL == sect]
            for b0 in range(0, len(ws), cfg.BPW):
                blocks.append((h, ws[b0:b0 + cfg.BPW]))
    # map window -> (block id, slot in block)
    win2blk = {}
    for bi, (h, ws) in enumerate(blocks):
        for si, w in enumerate(ws):
            win2blk[w] = (bi, si)
    lastwin2blk = {ws[-1]: bi for bi, (h, ws) in enumerate(blocks)}
    lastblk_of_half = {h: max(bi for bi, (hh, _) in enumerate(blocks) if hh == h)
                       for h in range(2)}

    # contiguous runs for building d_ar_chunk: chunk ci reads ar window
    # meta[ci][0]; a run is maximal with w advancing +1 per chunk
    ar_runs = []  # (chunk0, nchunks, w0)
    ci = 0
    while ci < NCH:
        w0 = meta[ci][0]
        n = 1
        while (ci + n < NCH and meta[ci + n][0] == meta[ci + n - 1][0] + 1
               and meta[ci + n][1]):
            n += 1
        ar_runs.append((ci, n, w0))
        ci += n
    # last chunk index of each window
    lastchunk = {}
    for ci, (w, first, last) in enumerate(meta):
        if last:
            lastchunk[w] = ci

    with tile.TileContext(nc) as tc:
        with (
            tc.tile_pool(name="const", bufs=1) as cp,
            tc.tile_pool(name="stage", bufs=3) as sp,
            tc.tile_pool(name="gath", bufs=cfg.GPB) as gp,
            tc.tile_pool(name="tt", bufs=3) as tp,
            tc.tile_pool(name="oh", bufs=4) as op,
            tc.tile_pool(name="small", bufs=4) as mp,
            tc.tile_pool(name="alrow", bufs=2) as rp,
            tc.tile_pool(name="arwin", bufs=3) as wp,
            tc.tile_pool(name="stgp", bufs=2) as stp,
            tc.tile_pool(name="psA", bufs=2, space="PSUM") as pp,      # misc + alT
            tc.tile_pool(name="psTT", bufs=2, space="PSUM") as pq,     # pstt [128,CS*DW]
            tc.tile_pool(name="psBLK", bufs=2, space="PSUM") as pb,    # blocks [128,512]
        ):
            # ---------- constants / persistent state
            w1 = cp.tile([128, cfg.KT, cfg.H], BF16, tag="w1")
            nc.sync.dma_start(out=w1[:], in_=t_w1t[:].rearrange("k p h -> p k h"))
            b1 = cp.tile([cfg.H, 1], F32, tag="b1")
            nc.sync.dma_start(out=b1[:], in_=t_b1[:])
            attb = cp.tile([128, cfg.NL * 2], BF16, tag="attb")
            nc.sync.dma_start(out=attb[:], in_=t_att[:])
            t2w = cp.tile([cfg.H, cfg.OUT], BF16, tag="t2w")
            nc.sync.dma_start(out=t2w[:], in_=t_t2[:])
            b2 = cp.tile([128, cfg.OUT], F32, tag="b2")
            nc.sync.dma_start(out=b2[:], in_=t_b2[:])
            iotab = cp.tile([128, cfg.DW], BF16, tag="iotab")
            nc.sync.dma_start(out=iotab[:], in_=t_iota[:])
            gidx = cp.tile([128, EPAD // 16], I16, tag="gidx")
            nc.sync.dma_start(out=gidx[:], in_=t_gidx[:])
            rel = cp.tile([128, NCH], F32, tag="rel")
            nc.sync.dma_start(out=rel[:], in_=t_rel[:])
            nrm = cp.tile([128, NCH], F32, tag="nrm")
            nc.sync.dma_start(out=nrm[:], in_=t_nrm[:])
            ones1b = cp.tile([1, 128], BF16, tag="ones1b")
            nc.vector.memset(ones1b[:], 1.0)
            onesDW = cp.tile([1, cfg.DW], BF16, tag="onesDW")
            nc.vector.memset(onesDW[:], 1.0)
            identb = cp.tile([128, 128], BF16, tag="identb")
            make_identity(nc, identb[:])

            # one-time zero init of the table: only needed under dbg
            # (CoreSim require_finite flags the never-computed junk in
            # row cols 129+ that the 512B gathers fetch)
            ztab = gp.tile([128, 8 * RWE], BF16, name="ztab", tag="ghs")
            if dbg:
                nc.vector.memset(ztab[:], 0.0)
            for dt_ in (d_tabs[:1] if dbg else []):
                for w0 in range(0, W, 8):
                    nb = min(8, W - w0)
                    nc.sync.dma_start(
                        out=dt_[w0 * 128:(w0 + nb) * 128, :].rearrange(
                            "(b p) c -> p b c", p=128),
                        in_=ztab[:, :nb * RWE].rearrange(
                            "p (b c) -> p b c", c=RWE))

            h_sb = cp.tile([128, cfg.NP], BF16, tag="h")
            raw_sb = cp.tile([128, cfg.NP], BF16, tag="raw")
            rs_sb = cp.tile([128, cfg.NP], SPDT, tag="rs")
            albx = cp.tile([128, W], BF16, tag="albx")
            arc = cp.tile([128, W], BF16, tag="arc")

            # ---------- per-window helper blocks
            TW = 8   # windows per merged table/output store
            hstage_cur = [None]

            def nprep_win(w, li, tab):
                """al/ar for window w of layer li, raw-h table rows."""
                sl = slice(w * 128, (w + 1) * 128)
                ps2 = pp.tile([128, 128], F32, tag="pg")
                nc.tensor.matmul(ps2[:, 0:2], lhsT=h_sb[:, sl],
                                 rhs=attb[:, 2 * li:2 * li + 2],
                                 start=True, stop=True)
                nc.vector.tensor_copy(albx[:, w:w + 1], ps2[:, 0:1])
                nc.vector.tensor_copy(arc[:, w:w + 1], ps2[:, 1:2])
                psT = pp.tile([128, 128], BF16, tag="pg")
                nc.tensor.transpose(out=psT[:], in_=h_sb[:, sl],
                                    identity=identb[:])
                if w % TW == 0:
                    hstage_cur[0] = sp.tile([128, TW * 128], BF16,
                                            name="hstage", tag="hst")
                hst = hstage_cur[0]
                wi = w % TW
                nc.scalar.activation(hst[:, wi * 128:(wi + 1) * 128], psT[:],
                                     mybir.ActivationFunctionType.Copy)
                if w % TW == TW - 1 or w == W - 1:
                    w0 = (w // TW) * TW
                    nb = w - w0 + 1
                    nc.sync.dma_start(
                        out=tab[w0 * 128:(w0 + nb) * 128, 0:cfg.H].rearrange(
                            "(b p) c -> p b c", p=128),
                        in_=hst[:, :nb * 128].rearrange(
                            "p (b c) -> p b c", c=128))

            runs_h = ([r for r in ar_runs
                       if (r[2] % cfg.WL) < cfg.WHA],
                      [r for r in ar_runs
                       if (r[2] % cfg.WL) >= cfg.WHA])

            def ar_publish(li, h, tab):
                """ar half h -> AllGather -> d_ar_chunk runs; at half B
                also store the al column into the table."""
                if h == 0:
                    lo, hi, wlo, whi = 0, cfg.NPHA, 0, cfg.WA128
                    d_full = d_ar_fullA
                else:
                    lo, hi, wlo, whi = cfg.NPHA, cfg.NP, cfg.WA128, W
                    d_full = d_ar_fullB
                    with nc.allow_non_contiguous_dma(reason="al-col store"):
                        nc.sync.dma_start(
                            out=tab[:, cfg.H:cfg.H + 1].rearrange(
                                "(t p) c -> p (t c)", p=128),
                            in_=albx[:])
                with nc.allow_non_contiguous_dma(reason="ar-col store"):
                    nc.sync.dma_start(
                        out=d_ar_loc[lo:hi].rearrange("(t p) -> p t", p=128),
                        in_=arc[:, wlo:whi])
                if "ag" not in skip:
                    nc.gpsimd.collective_compute(
                        "AllGather", mybir.AluOpType.bypass, replica_groups=rg,
                        ins=[d_ar_loc[lo:hi]], outs=[d_full[:]])
                for (ci0, n, w0) in runs_h[h]:
                    sect, wl = w0 // cfg.WL, w0 % cfg.WL
                    off = sect * NPHS[h] + (wl - (cfg.WHA if h else 0)) * DW
                    nc.sync.dma_start(
                        out=d_ar_chunk[ci0 * DW:(ci0 + n) * DW],
                        in_=d_full[off:off + n * DW])

            shw = cp.tile([128, W * cfg.OUT], BF16, tag="shw")
            smw = cp.tile([128, W], F32, tag="smw")

            embst_cur = [None]

            def phasec_win(t):
                sl = slice(t * 128, (t + 1) * 128)
                osl = slice(t * cfg.OUT, (t + 1) * cfg.OUT)
                pse = pp.tile([128, 128], F32, tag="pg")
                nc.tensor.matmul(pse[:, :cfg.OUT], lhsT=h_sb[:, sl], rhs=t2w[:],
                                 start=True, stop=True)
                if t % TW == 0:
                    embst_cur[0] = sp.tile([128, TW * cfg.OUT], F32,
                                           name="embst", tag="embst")
                embst = embst_cur[0]
                ti = t % TW
                esl = slice(ti * cfg.OUT, (ti + 1) * cfg.OUT)
                nc.vector.tensor_add(embst[:, esl], pse[:, :cfg.OUT], b2[:])
                if t % TW == TW - 1 or t == W - 1:
                    t0 = (t // TW) * TW
                    nb = t - t0 + 1
                    nc.sync.dma_start(
                        out=t_emb[t0 * 128:(t0 + nb) * 128, :].rearrange(
                            "(b p) c -> p b c", p=128),
                        in_=embst[:, :nb * cfg.OUT].rearrange(
                            "p (b c) -> p b c", c=cfg.OUT))
                mx = mp.tile([128, 1], F32, tag="mx")
                nc.vector.tensor_reduce(mx[:], embst[:, esl],
                                        axis=mybir.AxisListType.X,
                                        op=mybir.AluOpType.max)
                nc.vector.tensor_scalar(shw[:, osl], embst[:, esl], mx[:], None,
                                        op0=mybir.AluOpType.subtract)
                ex = sp.tile([128, cfg.OUT], F32, tag="ex")
                nc.scalar.activation(ex[:], shw[:, osl],
                                     mybir.ActivationFunctionType.Exp)
                nc.vector.tensor_reduce(smw[:, t:t + 1], ex[:],
                                        axis=mybir.AxisListType.X,
                                        op=mybir.AluOpType.add)

            def hnew_win(w):
                sl = slice(w * 128, (w + 1) * 128)
                nc.vector.tensor_add(h_sb[:, sl], rs_sb[:, sl], raw_sb[:, sl])

            def hdump(ix):
                if t_hdbg is not None:
                    hd = sp.tile([128, cfg.NP], F32, name="hd", tag="hd")
                    nc.vector.tensor_copy(hd[:], h_sb[:])
                    nc.sync.dma_start(out=t_hdbg[ix], in_=hd[:])
            hdump(0)

            # ---------- phase A: h = relu(x @ t1_w.T + b1)  (feature-major)
            AB = cfg.AB
            for t0 in range(0, W if "phasea" not in skip else 0, AB):
                nb = min(AB, W - t0)
                xa = gp.tile([128, AB * cfg.KT * 128], BF16, name="xa", tag="ghs")
                nc.sync.dma_start(
                    out=xa[:, :nb * cfg.KT * 128],
                    in_=t_xt[:, t0:t0 + nb].rearrange("p w k n -> p (w k n)"))
                for ti in range(nb):
                    t = t0 + ti
                    ps = pp.tile([128, 128], F32, tag="pg")
                    for k in range(cfg.KT):
                        o = (ti * cfg.KT + k) * 128
                        nc.tensor.matmul(ps[:], lhsT=w1[:, k, :],
                                         rhs=xa[:, o:o + 128],
                                         start=(k == 0), stop=(k == cfg.KT - 1))
                    nc.scalar.activation(h_sb[:, t * 128:(t + 1) * 128], ps[:],
                                         mybir.ActivationFunctionType.Relu,
                                         bias=b1[:])
                    nc.vector.tensor_scalar_mul(
                        raw_sb[:, t * 128:(t + 1) * 128],
                        h_sb[:, t * 128:(t + 1) * 128], EPS)
                    if "nprep" not in skip:
                        nprep_win(t, 0, d_tabs[0])
                        if t == cfg.WA128 - 1:
                            ar_publish(0, 0, d_tabs[0])
                        elif t == W - 1:
                            ar_publish(0, 1, d_tabs[0])

            # (layer-0 nprep rides inside phase A; publish per half)

            # ---------- layers
            for li in range(cfg.NL):
                tab_r = d_tabs[li % 2]
                tab_w = d_tabs[(li + 1) % 2]

                def emit_tail_win(w, li=li, tab_w=None):
                    hnew_win(w)
                    if li + 1 < cfg.NL:
                        if "nprep" not in skip:
                            nprep_win(w, li + 1, tab_w)
                            if w == cfg.WA128 - 1:
                                ar_publish(li + 1, 0, tab_w)
                            elif w == W - 1:
                                ar_publish(li + 1, 1, tab_w)
                    elif "phasec" not in skip:
                        phasec_win(w)

                rsA_seen = [False]
                rsA_call = [0]
                nextw = [0]
                if "edges" not in skip:
                    blk_tiles = {}
                    stg_cur = [None]
                    for ci_call, (c0, ncall) in enumerate(calls):
                        ghs = gp.tile([128, CS * RWE], BF16, tag="ghs")
                        ghv = ghs[:].rearrange("p (c e) -> p c e", e=RWE)
                        if "gather" not in skip:
                            GSC = 8   # chunks per gather sub-call (ring cap)
                            for s0 in range(0, ncall, GSC):
                                sn = min(GSC, ncall - s0)
                                ne = sn * 128
                                nc.gpsimd.dma_gather(
                                    out_ap=ghv[:, s0:s0 + sn, :],
                                    in_ap=tab_r[:],
                                    idxs_ap=gidx[:, (c0 + s0) * 8:
                                                 (c0 + s0 + sn) * 8],
                                    num_idxs=ne, num_idxs_reg=ne,
                                    elem_size=RWE,
                                    queue_num=(c0 // CS + s0 // GSC) % 2)
                        # al columns -> rows: PE transpose to PSUM, then DMA
                        # flatten onto partition 0 (matmul lhsT needs base 0)
                        alT = pp.tile([CS, 128], BF16, tag="alT")
                        nc.tensor.transpose(
                            out=alT[:ncall, :],
                            in_=ghv[:, :ncall, cfg.H],
                            identity=identb[:])
                        alR = mp.tile([CS, 128], BF16, tag="alR")
                        nc.scalar.activation(alR[:ncall, :], alT[:ncall, :],
                                             mybir.ActivationFunctionType.Copy)
                        alRow = rp.tile([1, CS * 128], BF16, tag="alRow")
                        nc.sync.dma_start(
                            out=alRow[0:1, :ncall * 128],
                            in_=alR[:ncall, :])
                        arWin = wp.tile([1, CS * DW], BF16, tag="arWin")
                        nc.sync.dma_start(
                            out=arWin[0:1, :ncall * DW],
                            in_=d_ar_chunk[c0 * DW:(c0 + ncall) * DW][None, :])
                        # pstt[e, j*DW+d] = al[e] + ar_win(j)[d], in
                        # groups of PSG chunks (PSUM bank budget), one
                        # wide tanh per group
                        PSG = 8
                        tt = tp.tile([128, CS * DW], BF16, tag="tt")
                        for g0 in range(0, ncall, PSG):
                            gn = min(PSG, ncall - g0)
                            pstt = pq.tile([128, PSG * DW], F32, tag="pstt")
                            for j in range(g0, g0 + gn):
                                jg = j - g0
                                nc.tensor.matmul(
                                    pstt[:, jg * DW:(jg + 1) * DW],
                                    lhsT=alRow[0:1, j * 128:(j + 1) * 128],
                                    rhs=onesDW[:],
                                    start=True, stop=False)
                                nc.tensor.matmul(
                                    pstt[:, jg * DW:(jg + 1) * DW],
                                    lhsT=ones1b[:],
                                    rhs=arWin[0:1, j * DW:(j + 1) * DW],
                                    start=False, stop=True)
                            nc.scalar.activation(
                                tt[:, g0 * DW:(g0 + gn) * DW],
                                pstt[:, :gn * DW],
                                mybir.ActivationFunctionType.Tanh)
                        # one-hot*norm per chunk, ohm merged per group
                        for g0 in range(0, ncall, PSG):
                            gn = min(PSG, ncall - g0)
                            ohp = op.tile([128, PSG * DW], BF16, tag="ohp")
                            for j in range(g0, g0 + gn):
                                cj = c0 + j
                                jg = j - g0
                                nc.vector.tensor_scalar(
                                    ohp[:, jg * DW:(jg + 1) * DW],
                                    iotab[:], rel[:, cj:cj + 1],
                                    nrm[:, cj:cj + 1],
                                    op0=mybir.AluOpType.is_equal,
                                    op1=mybir.AluOpType.mult)
                            ohm = op.tile([128, PSG * DW], BF16, tag="ohm")
                            nc.vector.tensor_tensor(
                                out=ohm[:, :gn * DW], in0=ohp[:, :gn * DW],
                                in1=tt[:, g0 * DW:(g0 + gn) * DW],
                                op=mybir.AluOpType.mult)
                            for j in range(g0, g0 + gn):
                                cj = c0 + j
                                jg = j - g0
                                w, first, last = meta[cj]
                                bi, si = win2blk[w]
                                if first and si == 0 and bi not in blk_tiles:
                                    blk_tiles[bi] = pb.tile(
                                        [128, cfg.BPW * DW], F32,
                                        name=f"blk{bi}", tag="blk")
                                blk = blk_tiles[bi]
                                nc.tensor.matmul(
                                    blk[:, si * DW:(si + 1) * DW],
                                    lhsT=ghv[:, j, 0:cfg.H],
                                    rhs=ohm[:, jg * DW:(jg + 1) * DW],
                                    start=first, stop=last)
                            # block complete -> evacuate PSUM to the
                            # section staging tile; section complete ->
                            # one spill DMA; half complete -> RS
                            if last and w in lastwin2blk:
                                bi2 = lastwin2blk[w]
                                hh, ws = blocks[bi2]
                                sect = ws[0] // cfg.WL
                                w0 = (ws[0] % cfg.WL) - hh * cfg.WH
                                ncol = len(ws) * DW
                                if w0 == 0:
                                    stg_cur[0] = sp.tile(
                                        [128, cfg.NPH], SPDT,
                                        name="stgsec", tag="stg")
                                stg = stg_cur[0]
                                nc.scalar.activation(
                                    stg[:, w0 * DW:w0 * DW + ncol],
                                    blk_tiles[bi2][:, :ncol],
                                    mybir.ActivationFunctionType.Copy)
                                del blk_tiles[bi2]
                                if w0 * DW + ncol == cfg.NPH:
                                    dpart = d_partA if hh == 0 else d_partB
                                    dview = dpart[:].rearrange(
                                        "(c p n) -> c p n", c=cfg.NC, p=128)
                                    nc.sync.dma_start(out=dview[sect, :, :],
                                                      in_=stg[:])
                                if (bi2 == lastblk_of_half[hh]
                                        and "rs" not in skip):
                                    dp, dr = ((d_partA, d_rsA) if hh == 0
                                              else (d_partB, d_rsB))
                                    nc.gpsimd.collective_compute(
                                        "ReduceScatter", mybir.AluOpType.add,
                                        replica_groups=rg,
                                        ins=[dp[:]], outs=[dr[:]])
                                    nc.sync.dma_start(
                                        out=rs_sb[:, hh * cfg.NPH:
                                                  (hh + 1) * cfg.NPH],
                                        in_=dr[:].rearrange(
                                            "(p n) -> p n", p=128))

                        # ride half-A h_new/nprep inside the half-B calls,
                        # starting only after RS-A has had time to drain
                        # (in-order queues: an op waiting on RS-A would
                        # stall everything behind it)
                        if rsA_seen[0] and "hnew" not in skip:
                            DELAY_CALLS = 10**9
                            remaining = len(calls) - ci_call - 1
                            ready = ci_call - rsA_call[0] >= DELAY_CALLS
                            if remaining == 0:
                                k = cfg.WA128 - nextw[0]
                            elif ready:
                                want = cfg.WA128 - nextw[0]
                                k = -(-want // remaining) if want > 0 else 0
                            else:
                                k = 0
                            for _ in range(min(k, cfg.WA128 - nextw[0])):
                                emit_tail_win(nextw[0], tab_w=tab_w)
                                nextw[0] += 1

                if dbg and li == 0 and t_tabdbg is not None:
                    nc.sync.dma_start(out=t_tabdbg[:], in_=tab_r[:])
                    nc.sync.dma_start(out=t_ardbg[:], in_=d_ar_chunk[:])
                    nc.sync.dma_start(out=t_padbg[:], in_=d_partA[:])
                    nc.sync.dma_start(out=t_pbdbg[:], in_=d_partB[:])
                    dbg_rs = sp.tile([128, cfg.NP], F32 if cfg.SPILL_F32
                                     else BF16, name="dbgrs", tag="dbgrs")
                    nc.vector.tensor_copy(dbg_rs[:], rs_sb[:])
                    nc.sync.dma_start(out=t_rsdbg[:], in_=dbg_rs[:])

                # remaining tail windows (half B; gated on RS-B)
                if "hnew" not in skip:
                    for w in range(nextw[0], W):
                        emit_tail_win(w, tab_w=tab_w)
                hdump(li + 1)

            # ---------- phase C epilogue
            if "phasec" not in skip:
                nc.scalar.activation(smw[:], smw[:],
                                     mybir.ActivationFunctionType.Ln)
                for t0 in range(0, W, TW):
                    nb = min(TW, W - t0)
                    lsm = sp.tile([128, TW * cfg.OUT], F32, tag="lsmt")
                    for ti in range(nb):
                        t = t0 + ti
                        nc.vector.tensor_scalar(
                            lsm[:, ti * cfg.OUT:(ti + 1) * cfg.OUT],
                            shw[:, t * cfg.OUT:(t + 1) * cfg.OUT],
                            smw[:, t:t + 1],
                            None, op0=mybir.AluOpType.subtract)
                    nc.sync.dma_start(
                        out=t_lsm[t0 * 128:(t0 + nb) * 128, :].rearrange(
                            "(b p) c -> p b c", p=128),
                        in_=lsm[:, :nb * cfg.OUT].rearrange(
                            "p (b c) -> p b c", c=cfg.OUT))

    nc.finalize()
    return nc


# ------------------------------------------------------- cached PJRT runner

def _make_runner(nc, n_cores):
    """Build the jitted SPMD executable once."""
    import jax
    import concourse.mybir as mb
    from jax.sharding import Mesh, PartitionSpec
    from jax.experimental.shard_map import shard_map
    from concourse.bass2jax import (install_neuronx_cc_hook, partition_id_tensor,
                                    _bass_exec_p)
    install_neuronx_cc_hook()
    partition_name = nc.partition_id_tensor.name if nc.partition_id_tensor else None
    in_names, out_names, out_avals, zero_outs = [], [], [], []
    for alloc in nc.m.functions[0].allocations:
        if not isinstance(alloc, mb.MemoryLocationSet):
            continue
        name = alloc.memorylocations[0].name
        if alloc.kind == "ExternalInput":
            if name != partition_name:
                in_names.append(name)
        elif alloc.kind == "ExternalOutput":
            out_names.append(name)
            shape = tuple(alloc.tensor_shape)
            dtype = mb.dt.np(alloc.dtype)
            out_avals.append(jax.core.ShapedArray(shape, dtype))
            zero_outs.append(np.zeros(shape, dtype))
    n_params = len(in_names)
    n_outs = len(out_avals)
    all_in_names = list(in_names) + list(out_names)
    if partition_name is not None:
        all_in_names.append(partition_name)
    donate = tuple(range(n_params, n_params + n_outs))

    def _body(*args):
        operands = list(args)
        if partition_name is not None:
            operands.append(partition_id_tensor())
        return tuple(_bass_exec_p.bind(
            *operands, out_avals=tuple(out_avals), in_names=tuple(all_in_names),
            out_names=tuple(out_names), lowering_input_output_aliases=(),
            sim_require_finite=True, sim_require_nnan=True, nc=nc))

    devices = jax.devices()[:n_cores]
    mesh = Mesh(np.asarray(devices), ("core",))
    in_specs = (PartitionSpec("core"),) * (n_params + n_outs)
    out_specs = (PartitionSpec("core"),) * n_outs
    sharded = jax.jit(
        shard_map(_body, mesh=mesh, in_specs=in_specs, out_specs=out_specs,
                  check_rep=False),
        donate_argnums=donate, keep_unused=True)

    def call(in_maps):
        concat_in = [
            np.concatenate([np.asarray(in_maps[c][k]) for c in range(n_cores)], 0)
            for k in in_names
        ]
        concat_zeros = [
            np.zeros((n_cores * z.shape[0], *z.shape[1:]), z.dtype)
            for z in zero_outs
        ]
        out_arrs = sharded(*concat_in, *concat_zeros)
        jax.block_until_ready(out_arrs)
        return [
            {k: np.asarray(out_arrs[i]).reshape(n_cores, *out_avals[i].shape)[c]
             for i, k in enumerate(out_names)}
            for c in range(n_cores)
        ]

    return call


# TimelineSim on the full program (collectives included); see t_sim.py.
HW_EXEC_NS_ESTIMATE = 1134300

# ----------------------------------------------------------------- entry

_CACHE = {}


def run(cfg, inputs, trace=False):
    ei = np.asarray(inputs["edge_index"])
    key = (cfg.N, cfg.E, cfg.NL, hash(ei.tobytes()))
    if key in _CACHE:
        runner, plan = _CACHE[key]
    else:
        plan = plan_edges(cfg, ei)
        nc = build_program(cfg, plan)
        runner = _make_runner(nc, cfg.NC)
        _CACHE[key] = (runner, plan)
    in_maps = shard_inputs(cfg, inputs, plan)
    results = runner(in_maps)
    lsm = np.concatenate([results[c]["lsm"][:cfg.NV] for c in range(cfg.NC)], 0)
    emb = np.concatenate([results[c]["emb"][:cfg.NV] for c in range(cfg.NC)], 0)
    return (lsm, emb), None


def kernel(**inputs):
    (lsm, emb), _ = run(FULL, inputs)
    return lsm, emb
